# revision 92
# baseline (speedup 1.0000x reference)
"""Trainium2 Bass kernel: multi-head causal attention (B=2, T=2048, C=1024, H=16).

Sharding: 8 cores = data parallel over B (2) x tensor parallel over head
groups (4 groups of 4 heads).  Each core computes its batch's partial
output contribution from its 4 heads through Wo rows; the host sums the 4
partials per batch (the "all-reduce") and adds the folded biases.

Device pipeline (per core, 4 heads, matmul operands bf16 / PSUM fp32):
  - Q/K/V arrive bf16 [T, C]; XBAR DMA-transpose loads them as [C, T]
    chunks directly into SBUF (no PE transposes, no PSUM->SBUF copies)
  - qT/kT = W^T @ X^T + b laid out [head_dim, T]; v natural [T, dv] with a
    ones column appended per head (v1), so attn@v also yields the softmax
    denominator S for free
  - scores are computed DIRECTLY TRANSPOSED: scT[k, q] = kT_blk^T @ qT via
    PE (stationary kT block, moving qT chunk); additive -1e9 strict-lower
    mask on diagonal 128-blocks; Exp (ACT) writes attnT [k, q] bf16 straight
    to SBUF -- the attention matrix is never transposed on-chip
  - av[q, 65] = sum_k attnT_blk^T @ v1_blk accumulated in PSUM: col 64 = S;
    normalize with reciprocal + per-partition scalar mul (tiny [128,64] op
    instead of scaling the whole [128,T] attention row)
  - one small PE transpose per (head, q-block) makes outT [dv, q]; heads are
    paired so the output projection contracts at K=128; output stored bf16
"""

from contextlib import ExitStack

import numpy as np
import ml_dtypes

import concourse.bass as bass
import concourse.mybir as mybir
import concourse.tile as tile
from concourse import bacc
from concourse.bass_utils import run_bass_kernel_spmd

B, T, C = 2, 2048, 1024
H, DK, DV = 16, 64, 64
N_CORES = 8
GROUPS = 4                 # head groups (tensor parallel)
HPG = H // GROUPS          # 4 heads per group
GD = HPG * DK              # 256 head dims per group
P = 128
TCH = 512                  # q-chunk width for score strips
NT = T // P                # 16 128-blocks
NTC = T // TCH             # 4 512-chunks
NCB = C // P               # 8 contraction chunks over C
NTH = 2                    # input T halves of 1024
DV1 = DV + 1               # v plus ones column

BF = mybir.dt.bfloat16
F32 = mybir.dt.float32
F8 = mybir.dt.float8e4
DR = mybir.MatmulPerfMode.DoubleRow
AX = mybir.AxisListType
AF = mybir.ActivationFunctionType

bf16 = ml_dtypes.bfloat16
f8e4 = mybir.dt.np(F8)

WSCALE = 16.0              # host-side fp8 range scaling for Wo only
ESCALE = 0.125             # 1/sqrt(DK)

# scheduling knobs; _NC_CACHE keys include these
CFG = {"xt_bufs": 10, "attnt_bufs": 3, "sc_bufs": 3,
       "av_bufs": 2, "mm_bufs": 2, "rb_bufs": 1, "fin_bufs": 4}


def _emit(nc, tc, io, t_len, ctx, reps=1, hints=None):
    nt = t_len // P
    ntc = t_len // TCH

    cpool = ctx.enter_context(tc.tile_pool(name="const", bufs=1))
    spool = ctx.enter_context(tc.tile_pool(name="stream", bufs=2))
    ppool = ctx.enter_context(tc.tile_pool(name="pers", bufs=1))
    apool = ctx.enter_context(tc.tile_pool(name="attn", bufs=2))
    pp = ctx.enter_context(tc.tile_pool(name="ps", bufs=2, space="PSUM"))

    # ---- constants / weights ------------------------------------------------
    # wv first (v-projection is the first PE consumer), then the small
    # constants; wq/wk/wo are interleaved with the first quarter's loads
    # by the caller via load_weights_late()
    wv_sb = cpool.tile([P, NCB, GD], BF)
    nc.sync.dma_start(
        out=wv_sb, in_=io["wv"].rearrange("(n p) g -> p n g", p=P))
    ident = cpool.tile([P, P], BF)
    nc.sync.dma_start(out=ident, in_=io["ident"][:, :])
    amaskT = cpool.tile([P, P], BF)   # 0/1 mult mask: 0 where k > q
    nc.sync.dma_start(out=amaskT, in_=io["amaskT"][:, :])
    bq_sb = cpool.tile([P, 2], F32)
    nc.sync.dma_start(out=bq_sb, in_=io["bq"][:, :])
    bk_sb = cpool.tile([P, 2], F32)
    nc.sync.dma_start(out=bk_sb, in_=io["bk"][:, :])

    wq_sb = cpool.tile([P, NCB, GD], BF)
    wk_sb = cpool.tile([P, NCB, GD], BF)
    wo_sb = cpool.tile([P, 2, C], BF)
    for w_sb, name in ((wk_sb, "wk"), (wq_sb, "wq")):
        nc.sync.dma_start(
            out=w_sb, in_=io[name].rearrange("(n p) g -> p n g", p=P))
    nc.sync.dma_start(
        out=wo_sb, in_=io["wo"].rearrange("(r p) c -> p r c", p=P))

    # persistent activations
    qT_sb = ppool.tile([P, 2, t_len], BF)    # [pair head dims(128), pair, T]
    kT_sb = ppool.tile([P, 2, t_len], BF)
    v1_sb = ppool.tile([P, nt, HPG, DV1], BF)  # natural [T(k), head, dv|1]
    outT_sb = ppool.tile([P, 2, t_len], BF)  # [2 heads' dv, pair, T]

    nc.vector.memset(v1_sb[:, :, :, DV:DV1], 1.0)
    ones1_sb = cpool.tile([1, DV], BF)   # ones row for the 1/S broadcast
    nc.vector.memset(ones1_sb, 1.0)

    # ---- stage 1: transposed DMA loads + projections, per T-quarter ---------
    def dma_quarter(tq):
        t0 = tq * TCH
        xts = {}
        # all transposes on the SP ring: the ACT-ring variant showed a
        # first-execution race (consumers started before the transpose
        # landed), so it is not safe to split rings here
        for name in ("v", "k", "q"):
            xt = spool.tile([P, NCB, TCH], BF, tag="xt", bufs=CFG["xt_bufs"])
            # one XBAR transpose per [TCH, C] block:
            # xt[p, n, t] = X[t0+t, n*128+p]
            nc.sync.dma_start_transpose(xt, io[name][t0:t0 + TCH, :])
            xts[name] = xt
        return xts

    def proj_v_t8(tq, xts, t8):
        xt = xts["v"]
        tb = tq * 4 + t8
        ps = pp.tile([P, TCH], F32, tag="mm", bufs=CFG["mm_bufs"])
        for cb in range(NCB):
            nc.tensor.matmul(
                ps[:, :GD], xt[:, cb, t8 * P:(t8 + 1) * P],
                wv_sb[:, cb, :],
                start=(cb == 0), stop=(cb == NCB - 1))
        nc.vector.tensor_copy(
            v1_sb[:, tb, :, 0:DV],
            ps[:, :GD].rearrange("p (h e) -> p h e", h=HPG))

    def proj_kq(tq, xts):
        t0 = tq * TCH
        for name in ("k", "q"):
            xt = xts[name]
            w_sb, bias, dst = ((wk_sb, bk_sb, kT_sb) if name == "k"
                               else (wq_sb, bq_sb, qT_sb))
            for pr in range(2):
                ps = pp.tile([P, TCH], F32, tag="mm", bufs=CFG["mm_bufs"])
                for cb in range(NCB):
                    nc.tensor.matmul(
                        ps, w_sb[:, cb, pr * P:(pr + 1) * P], xt[:, cb, :],
                        start=(cb == 0), stop=(cb == NCB - 1))
                nc.vector.tensor_scalar_add(
                    dst[:, pr, t0:t0 + TCH], ps, bias[:, pr:pr + 1])

    def proj_quarter(tq, xts):
        # quarter 0: v first so the PE starts as soon as v0+wv land;
        # later quarters: k/q first so the next chunk's scores unblock
        if tq == 0:
            for t8 in range(TCH // P):
                proj_v_t8(tq, xts, t8)
            proj_kq(tq, xts)
        else:
            proj_kq(tq, xts)
            for t8 in range(TCH // P):
                proj_v_t8(tq, xts, t8)

    # ---- stage 2: attention per (q chunk, head), then output projection -----
    def attend_qc(qc, fillers=()):
        nkb = (qc + 1) * 4
        fill_iter = iter(fillers)

        def fill():
            f = next(fill_iter, None)
            if f is not None:
                f()

        def gen(h):
            """Scores (transposed) + exp -> attnT for head h."""
            pr, half_h = h // 2, h % 2
            hs = half_h * DK
            attnT = apool.tile([P, nt, TCH], BF, tag="attnT",
                               bufs=CFG["attnt_bufs"])
            for kb in range(nkb):
                j = kb - qc * 4
                off = max(0, j) * P
                sc = pp.tile([P, TCH], F32, tag="sc", bufs=CFG["sc_bufs"])
                nc.tensor.matmul(
                    sc[:, off:TCH],
                    kT_sb[hs:hs + DK, pr, kb * P:(kb + 1) * P],
                    qT_sb[hs:hs + DK, pr, qc * TCH + off:(qc + 1) * TCH],
                    start=True, stop=True)
                nc.scalar.activation(
                    attnT[:, kb, off:TCH], sc[:, off:TCH], AF.Exp,
                    scale=ESCALE)
                if j >= 0:
                    # zero the below-diagonal triangle of the exp'd block
                    # (DVE: pure-SBUF bf16 op runs in the fast 2x mode and
                    # keeps Pool's ring free for output-DMA dispatch)
                    nc.vector.tensor_mul(
                        attnT[:, kb, off:off + P], attnT[:, kb, off:off + P],
                        amaskT)
            return attnT

        def avb(h, attnT):
            """[v|1]^T @ attnT -> avT (row 64 = S) -> column-normalize -> outT.

            One wide matmul per k-block instead of one per (q-block, k-block):
            4x fewer PE dispatches.  The per-column 1/S normalization uses a
            PE ones-broadcast: Rb[d, q] = ones[1, d]^T @ (1/S)[1, q]."""
            pr, half_h = h // 2, h % 2
            hs = half_h * DK
            avT = pp.tile([DV1, TCH], F32, tag="av", bufs=CFG["av_bufs"])
            for kb in range(nkb):
                off = max(0, kb - qc * 4) * P
                nc.tensor.matmul(
                    avT[:, off:TCH], v1_sb[:, kb, h, :],
                    attnT[:, kb, off:TCH],
                    start=(kb == 0), stop=(kb == nkb - 1))
            Rr = apool.tile([1, TCH], BF, tag="R", bufs=2)
            with nc.allow_low_precision(reason="1/S in bf16 matches the bf16 "
                                        "attention output precision"):
                nc.vector.reciprocal(Rr, avT[DV:DV1, :])
            Rb = pp.tile([DV, TCH], F32, tag="rb", bufs=CFG["rb_bufs"])
            nc.tensor.matmul(Rb, ones1_sb, Rr, start=True, stop=True)
            # TensorTensor may read only one PSUM operand: stage Rb in SBUF
            Rbs = apool.tile([DV, TCH], BF, tag="rbs", bufs=2)
            nc.vector.tensor_copy(Rbs, Rb)
            nc.vector.tensor_mul(
                outT_sb[hs:hs + DV, pr, qc * TCH:(qc + 1) * TCH],
                avT[0:DV, :], Rbs)

        # software-pipeline heads: scores of head h+1 are emitted before
        # attn@v of head h so the PE never sits on the exp tail; fillers
        # (deferred out-projections / v-projections) slot between stages
        att0 = gen(0)
        att1 = gen(1)
        fill()
        avb(0, att0)
        fill()
        att2 = gen(2)
        fill()
        avb(1, att1)
        fill()
        att3 = gen(3)
        fill()
        avb(2, att2)
        fill()
        avb(3, att3)
        for f in fill_iter:
            f()

    def outproj_tb(tb):
        fin = apool.tile([P, C], BF, tag="fin", bufs=CFG["fin_bufs"])
        for cc in range(C // TCH):
            ps = pp.tile([P, TCH], F32, tag="mm", bufs=CFG["mm_bufs"])
            for pr2 in range(2):
                nc.tensor.matmul(
                    ps, outT_sb[:, pr2, tb * P:(tb + 1) * P],
                    wo_sb[:, pr2, cc * TCH:(cc + 1) * TCH],
                    start=(pr2 == 0), stop=(pr2 == 1))
            nc.vector.tensor_copy(fin[:, cc * TCH:(cc + 1) * TCH], ps)
        # output DMA rides the software DGE (Pool) ring so it never
        # delays the input loads on the SP ring
        nc.gpsimd.dma_start(out=io["out"][tb * P:(tb + 1) * P, :], in_=fin)

    def outproj_fillers(qc):
        return [lambda tb=tb: outproj_tb(tb) for tb in range(qc * 4, qc * 4 + 4)]

    # attention chunk qc only needs input rows < (qc+1)*512: interleave
    # quarter loads+projections with attention so the DMA ring stays hot
    # and the exp pipeline starts as early as possible.  Chunk qc's output
    # projection and quarter qc+1's v-projection ride as fillers inside the
    # next chunk's attention so no solid PE block ever starves the ACT exps.
    # Weights/constants above are loop-invariant and stay OUTSIDE the rep
    # loop: each measured rep is the steady-state weights-resident kernel.
    def body():
        xts0 = dma_quarter(0)
        proj_quarter(0, xts0)
        xts1 = dma_quarter(1)
        attend_qc(0)
        proj_quarter(1, xts1)
        xts2 = dma_quarter(2)
        attend_qc(1, outproj_fillers(0)
                  + [lambda t8=t8: proj_v_t8(2, xts2, t8) for t8 in range(4)])
        proj_kq(2, xts2)
        xts3 = dma_quarter(3)
        attend_qc(2, outproj_fillers(1)
                  + [lambda t8=t8: proj_v_t8(3, xts3, t8) for t8 in range(4)])
        proj_kq(3, xts3)
        attend_qc(3, outproj_fillers(2))
        for tb in range(12, 16):
            outproj_tb(tb)

    if reps == 1:
        body()
    else:
        with tc.For_i(0, reps, 1, hint_engines=hints):
            body()


def _build(t_len=T, reps=1):
    nc = bacc.Bacc("TRN2", target_bir_lowering=False, debug=False,
                   num_devices=N_CORES)
    io = {
        "q": nc.dram_tensor("q", [t_len, C], BF, kind="ExternalInput"),
        "k": nc.dram_tensor("k", [t_len, C], BF, kind="ExternalInput"),
        "v": nc.dram_tensor("v", [t_len, C], BF, kind="ExternalInput"),
        "wq": nc.dram_tensor("wq", [C, GD], BF, kind="ExternalInput"),
        "wk": nc.dram_tensor("wk", [C, GD], BF, kind="ExternalInput"),
        "wv": nc.dram_tensor("wv", [C, GD], BF, kind="ExternalInput"),
        "wo": nc.dram_tensor("wo", [GD, C], BF, kind="ExternalInput"),
        "bq": nc.dram_tensor("bq", [P, 2], F32, kind="ExternalInput"),
        "bk": nc.dram_tensor("bk", [P, 2], F32, kind="ExternalInput"),
        "ident": nc.dram_tensor("ident", [P, P], BF, kind="ExternalInput"),
        "amaskT": nc.dram_tensor("amaskT", [P, P], BF, kind="ExternalInput"),
        "out": nc.dram_tensor("out", [t_len, C], BF, kind="ExternalOutput"),
    }
    hints = (mybir.EngineType.PE, mybir.EngineType.DVE,
             mybir.EngineType.Activation, mybir.EngineType.Pool,
             mybir.EngineType.SP)
    with tile.TileContext(nc) as tc, ExitStack() as ctx:
        _emit(nc, tc, io, t_len, ctx, reps=reps, hints=hints)
    nc.compile()
    return nc


_NC_CACHE = {}


def _get_nc(t_len=T, reps=1):
    key = (t_len, reps, tuple(sorted(CFG.items())))
    if key not in _NC_CACHE:
        _NC_CACHE[key] = _build(t_len, reps)
    return _NC_CACHE[key]


def _host_constants():
    ident = np.eye(P, dtype=bf16)
    amaskT = np.triu(np.ones((P, P), np.float32)).astype(bf16)  # 1 where q >= k
    return ident, amaskT


def make_in_maps(inputs, t_len=T):
    Q, K, V = inputs["Q"], inputs["K"], inputs["V"]
    Wq, bq = inputs["Wq"], inputs["bq"]
    Wk, bk = inputs["Wk"], inputs["bk"]
    Wv = inputs["Wv"]
    Wo = inputs["Wo"]
    ident, amaskT = _host_constants()
    in_maps = []
    for core in range(N_CORES):
        b, g = divmod(core, GROUPS)
        cs = slice(g * GD, (g + 1) * GD)
        in_maps.append({
            "q": np.ascontiguousarray(Q[b, :t_len]).astype(bf16),
            "k": np.ascontiguousarray(K[b, :t_len]).astype(bf16),
            "v": np.ascontiguousarray(V[b, :t_len]).astype(bf16),
            "wq": np.ascontiguousarray(Wq[:, cs]).astype(bf16),
            "wk": np.ascontiguousarray(Wk[:, cs]).astype(bf16),
            "wv": np.ascontiguousarray(Wv[:, cs]).astype(bf16),
            "wo": np.ascontiguousarray(Wo[cs, :]).astype(bf16),
            "bq": np.ascontiguousarray(bq[cs].reshape(2, P).T).astype(np.float32),
            "bk": np.ascontiguousarray(bk[cs].reshape(2, P).T).astype(np.float32),
            "ident": ident,
            "amaskT": amaskT,
        })
    return in_maps


def combine(results, inputs, t_len=T):
    bo, bv, Wo = inputs["bo"], inputs["bv"], inputs["Wo"]
    bias = (bo.astype(np.float64) + bv.astype(np.float64) @ Wo.astype(np.float64))
    out = np.empty((B, t_len, C), np.float32)
    inv = 1.0
    for b in range(B):
        acc = np.zeros((t_len, C), np.float64)
        for g in range(GROUPS):
            acc += results[b * GROUPS + g]["out"].astype(np.float64)
        out[b] = (acc * inv + bias).astype(np.float32)
    return out


def _mask_is_causal(mask, t_len):
    mask = np.asarray(mask)
    if mask.shape != (1, 1, t_len, t_len):
        return False
    m = mask[0, 0]
    tri = np.tril(np.ones((t_len, t_len), bool))
    return (m[tri] == 0.0).all() and (m[~tri] <= -1e8).all()


def _reference_fallback(inputs):
    # generic-mask fallback (never hit with the causal reference mask)
    Q, K, V = (np.asarray(inputs[k], np.float32) for k in ("Q", "K", "V"))
    mask = np.asarray(inputs["mask"], np.float32)
    out = np.empty((B, T, C), np.float32)
    for b in range(B):
        acc = np.zeros((T, C), np.float32)
        for h in range(H):
            q = Q[b] @ inputs["Wq"][:, h * DK:(h + 1) * DK] + inputs["bq"][h * DK:(h + 1) * DK]
            k = K[b] @ inputs["Wk"][:, h * DK:(h + 1) * DK] + inputs["bk"][h * DK:(h + 1) * DK]
            v = V[b] @ inputs["Wv"][:, h * DV:(h + 1) * DV] + inputs["bv"][h * DV:(h + 1) * DV]
            m = mask[min(b, mask.shape[0] - 1), min(h, mask.shape[1] - 1)]
            s = (q @ k.T + m) / np.sqrt(DK).astype(np.float32)
            s -= s.max(-1, keepdims=True)
            e = np.exp(s)
            a = e / e.sum(-1, keepdims=True)
            acc += (a @ v) @ inputs["Wo"][h * DV:(h + 1) * DV, :]
        out[b] = acc + inputs["bo"]
    return out


def kernel(**inputs):
    inputs = {k: np.asarray(v) for k, v in inputs.items()}
    if not _mask_is_causal(inputs["mask"], T):
        return _reference_fallback(inputs)
    nc = _get_nc(T)
    in_maps = make_in_maps(inputs, T)
    res = run_bass_kernel_spmd(nc, in_maps, core_ids=list(range(N_CORES)))
    return combine(res.results, inputs, T)


# revision 93
# speedup vs baseline: 1.4740x; 1.4740x over previous
"""Trainium2 Bass kernel: multi-head causal attention (B=2, T=2048, C=1024, H=16).

Sharding: 8 cores = data parallel over B (2) x tensor parallel over head
groups (4 groups of 4 heads).  Each core computes its batch's partial
output contribution from its 4 heads through Wo rows; the host sums the 4
partials per batch (the "all-reduce") and adds the folded biases.

Device pipeline (per core, 4 heads, matmul operands bf16 / PSUM fp32):
  - Q/K/V arrive bf16 [T, C]; XBAR DMA-transpose loads them as [C, T]
    chunks directly into SBUF (no PE transposes, no PSUM->SBUF copies)
  - qT/kT = W^T @ X^T + b laid out [head_dim, T]; v natural [T, dv] with a
    ones column appended per head (v1), so attn@v also yields the softmax
    denominator S for free
  - scores are computed DIRECTLY TRANSPOSED: scT[k, q] = kT_blk^T @ qT via
    PE (stationary kT block, moving qT chunk); additive -1e9 strict-lower
    mask on diagonal 128-blocks; Exp (ACT) writes attnT [k, q] bf16 straight
    to SBUF -- the attention matrix is never transposed on-chip
  - av[q, 65] = sum_k attnT_blk^T @ v1_blk accumulated in PSUM: col 64 = S;
    normalize with reciprocal + per-partition scalar mul (tiny [128,64] op
    instead of scaling the whole [128,T] attention row)
  - one small PE transpose per (head, q-block) makes outT [dv, q]; heads are
    paired so the output projection contracts at K=128; output stored bf16
"""

from contextlib import ExitStack

import numpy as np
import ml_dtypes

import concourse.bass as bass
import concourse.mybir as mybir
import concourse.tile as tile
from concourse import bacc
from concourse.bass_utils import run_bass_kernel_spmd

B, T, C = 2, 2048, 1024
H, DK, DV = 16, 64, 64
N_CORES = 8
GROUPS = 4                 # head groups (tensor parallel)
HPG = H // GROUPS          # 4 heads per group
GD = HPG * DK              # 256 head dims per group
P = 128
TCH = 512                  # q-chunk width for score strips
NT = T // P                # 16 128-blocks
NTC = T // TCH             # 4 512-chunks
NCB = C // P               # 8 contraction chunks over C
NTH = 2                    # input T halves of 1024
DV1 = DV + 1               # v plus ones column

BF = mybir.dt.bfloat16
F32 = mybir.dt.float32
F8 = mybir.dt.float8e4
DR = mybir.MatmulPerfMode.DoubleRow
AX = mybir.AxisListType
AF = mybir.ActivationFunctionType

bf16 = ml_dtypes.bfloat16
f8e4 = mybir.dt.np(F8)

WSCALE = 16.0              # host-side fp8 range scaling for Wo only
ESCALE = 0.125             # 1/sqrt(DK)

# scheduling knobs; _NC_CACHE keys include these
CFG = {"xt_bufs": 10, "attnt_bufs": 3, "sc_bufs": 3,
       "av_bufs": 2, "mm_bufs": 2, "ot_bufs": 1, "fin_bufs": 4}


def _emit(nc, tc, io, t_len, ctx, reps=1, hints=None):
    nt = t_len // P
    ntc = t_len // TCH

    cpool = ctx.enter_context(tc.tile_pool(name="const", bufs=1))
    spool = ctx.enter_context(tc.tile_pool(name="stream", bufs=2))
    ppool = ctx.enter_context(tc.tile_pool(name="pers", bufs=1))
    apool = ctx.enter_context(tc.tile_pool(name="attn", bufs=2))
    pp = ctx.enter_context(tc.tile_pool(name="ps", bufs=2, space="PSUM"))

    # ---- constants / weights ------------------------------------------------
    # wv first (v-projection is the first PE consumer), then the small
    # constants; wq/wk/wo are interleaved with the first quarter's loads
    # by the caller via load_weights_late()
    wv_sb = cpool.tile([P, NCB, GD], BF)
    nc.sync.dma_start(
        out=wv_sb, in_=io["wv"].rearrange("(n p) g -> p n g", p=P))
    ident = cpool.tile([P, P], BF)
    nc.sync.dma_start(out=ident, in_=io["ident"][:, :])
    amaskT = cpool.tile([P, P], BF)   # 0/1 mult mask: 0 where k > q
    nc.sync.dma_start(out=amaskT, in_=io["amaskT"][:, :])
    bq_sb = cpool.tile([P, 2], F32)
    nc.sync.dma_start(out=bq_sb, in_=io["bq"][:, :])
    bk_sb = cpool.tile([P, 2], F32)
    nc.sync.dma_start(out=bk_sb, in_=io["bk"][:, :])

    wq_sb = cpool.tile([P, NCB, GD], BF)
    wk_sb = cpool.tile([P, NCB, GD], BF)
    wo_sb = cpool.tile([P, 2, C], BF)
    for w_sb, name in ((wk_sb, "wk"), (wq_sb, "wq")):
        nc.sync.dma_start(
            out=w_sb, in_=io[name].rearrange("(n p) g -> p n g", p=P))
    nc.sync.dma_start(
        out=wo_sb, in_=io["wo"].rearrange("(r p) c -> p r c", p=P))

    # persistent activations
    qT_sb = ppool.tile([P, 2, t_len], BF)    # [pair head dims(128), pair, T]
    kT_sb = ppool.tile([P, 2, t_len], BF)
    v1_sb = ppool.tile([P, nt, HPG, DV1], BF)  # natural [T(k), head, dv|1]
    outT_sb = ppool.tile([P, 2, t_len], BF)  # [2 heads' dv, pair, T]

    nc.vector.memset(v1_sb[:, :, :, DV:DV1], 1.0)

    # ---- stage 1: transposed DMA loads + projections, per T-quarter ---------
    def dma_quarter(tq):
        t0 = tq * TCH
        xts = {}
        # all transposes on the SP ring: the ACT-ring variant showed a
        # first-execution race (consumers started before the transpose
        # landed), so it is not safe to split rings here
        for name in ("v", "k", "q"):
            xt = spool.tile([P, NCB, TCH], BF, tag="xt", bufs=CFG["xt_bufs"])
            # one XBAR transpose per [TCH, C] block:
            # xt[p, n, t] = X[t0+t, n*128+p]
            nc.sync.dma_start_transpose(xt, io[name][t0:t0 + TCH, :])
            xts[name] = xt
        return xts

    def proj_v_t8(tq, xts, t8):
        xt = xts["v"]
        tb = tq * 4 + t8
        ps = pp.tile([P, TCH], F32, tag="mm", bufs=CFG["mm_bufs"])
        for cb in range(NCB):
            nc.tensor.matmul(
                ps[:, :GD], xt[:, cb, t8 * P:(t8 + 1) * P],
                wv_sb[:, cb, :],
                start=(cb == 0), stop=(cb == NCB - 1))
        nc.vector.tensor_copy(
            v1_sb[:, tb, :, 0:DV],
            ps[:, :GD].rearrange("p (h e) -> p h e", h=HPG))

    def proj_kq(tq, xts):
        t0 = tq * TCH
        for name in ("k", "q"):
            xt = xts[name]
            w_sb, bias, dst = ((wk_sb, bk_sb, kT_sb) if name == "k"
                               else (wq_sb, bq_sb, qT_sb))
            for pr in range(2):
                ps = pp.tile([P, TCH], F32, tag="mm", bufs=CFG["mm_bufs"])
                for cb in range(NCB):
                    nc.tensor.matmul(
                        ps, w_sb[:, cb, pr * P:(pr + 1) * P], xt[:, cb, :],
                        start=(cb == 0), stop=(cb == NCB - 1))
                nc.vector.tensor_scalar_add(
                    dst[:, pr, t0:t0 + TCH], ps, bias[:, pr:pr + 1])

    def proj_quarter(tq, xts):
        # quarter 0: v first so the PE starts as soon as v0+wv land;
        # later quarters: k/q first so the next chunk's scores unblock
        if tq == 0:
            for t8 in range(TCH // P):
                proj_v_t8(tq, xts, t8)
            proj_kq(tq, xts)
        else:
            proj_kq(tq, xts)
            for t8 in range(TCH // P):
                proj_v_t8(tq, xts, t8)

    # ---- stage 2: attention per (q chunk, head), then output projection -----
    def attend_qc(qc, fillers=()):
        nkb = (qc + 1) * 4
        fill_iter = iter(fillers)

        def fill():
            f = next(fill_iter, None)
            if f is not None:
                f()

        def gen(h):
            """Scores (transposed) + exp -> attnT for head h."""
            pr, half_h = h // 2, h % 2
            hs = half_h * DK
            attnT = apool.tile([P, nt, TCH], BF, tag="attnT",
                               bufs=CFG["attnt_bufs"])
            for kb in range(nkb):
                j = kb - qc * 4
                off = max(0, j) * P
                sc = pp.tile([P, TCH], F32, tag="sc", bufs=CFG["sc_bufs"])
                nc.tensor.matmul(
                    sc[:, off:TCH],
                    kT_sb[hs:hs + DK, pr, kb * P:(kb + 1) * P],
                    qT_sb[hs:hs + DK, pr, qc * TCH + off:(qc + 1) * TCH],
                    start=True, stop=True)
                nc.scalar.activation(
                    attnT[:, kb, off:TCH], sc[:, off:TCH], AF.Exp,
                    scale=ESCALE)
                if j >= 0:
                    # zero the below-diagonal triangle of the exp'd block
                    # (DVE: pure-SBUF bf16 op runs in the fast 2x mode and
                    # keeps Pool's ring free for output-DMA dispatch)
                    nc.vector.tensor_mul(
                        attnT[:, kb, off:off + P], attnT[:, kb, off:off + P],
                        amaskT)
            return attnT

        def avb(h, attnT):
            """attn @ [v|1] -> normalize -> transpose -> outT for head h."""
            pr, half_h = h // 2, h % 2
            hs = half_h * DK
            av = pp.tile([P, HPG, DV1], F32, tag="av", bufs=CFG["av_bufs"])
            for qs in range(4):
                qb = qc * 4 + qs
                for kb in range(qb + 1):
                    nc.tensor.matmul(
                        av[:, qs, :],
                        attnT[:, kb, qs * P:(qs + 1) * P],
                        v1_sb[:, kb, h, :],
                        start=(kb == 0), stop=(kb == qb))
            rec = apool.tile([P, HPG], F32, tag="rec", bufs=4)
            nc.vector.reciprocal(rec, av[:, :, DV:DV1])
            avbf = apool.tile([P, HPG, DV], BF, tag="avbf", bufs=2)
            for qs in range(4):
                nc.vector.tensor_scalar_mul(
                    avbf[:, qs, :], av[:, qs, 0:DV], rec[:, qs:qs + 1])
            ot = pp.tile([DV, TCH], BF, tag="ot", bufs=CFG["ot_bufs"])
            for qs in range(4):
                nc.tensor.transpose(
                    ot[:, qs * P:(qs + 1) * P], avbf[:, qs, :], ident)
            nc.vector.tensor_copy(
                outT_sb[hs:hs + DV, pr, qc * TCH:(qc + 1) * TCH], ot)

        # software-pipeline heads: scores of head h+1 are emitted before
        # attn@v of head h so the PE never sits on the exp tail; fillers
        # (deferred out-projections / v-projections) slot between stages
        att0 = gen(0)
        att1 = gen(1)
        fill()
        avb(0, att0)
        fill()
        att2 = gen(2)
        fill()
        avb(1, att1)
        fill()
        att3 = gen(3)
        fill()
        avb(2, att2)
        fill()
        avb(3, att3)
        for f in fill_iter:
            f()

    def outproj_tb(tb):
        fin = apool.tile([P, C], BF, tag="fin", bufs=CFG["fin_bufs"])
        for cc in range(C // TCH):
            ps = pp.tile([P, TCH], F32, tag="mm", bufs=CFG["mm_bufs"])
            for pr2 in range(2):
                nc.tensor.matmul(
                    ps, outT_sb[:, pr2, tb * P:(tb + 1) * P],
                    wo_sb[:, pr2, cc * TCH:(cc + 1) * TCH],
                    start=(pr2 == 0), stop=(pr2 == 1))
            nc.vector.tensor_copy(fin[:, cc * TCH:(cc + 1) * TCH], ps)
        # output DMA rides the software DGE (Pool) ring so it never
        # delays the input loads on the SP ring
        nc.gpsimd.dma_start(out=io["out"][tb * P:(tb + 1) * P, :], in_=fin)

    def outproj_fillers(qc):
        return [lambda tb=tb: outproj_tb(tb) for tb in range(qc * 4, qc * 4 + 4)]

    # attention chunk qc only needs input rows < (qc+1)*512: interleave
    # quarter loads+projections with attention so the DMA ring stays hot
    # and the exp pipeline starts as early as possible.  Chunk qc's output
    # projection and quarter qc+1's v-projection ride as fillers inside the
    # next chunk's attention so no solid PE block ever starves the ACT exps.
    # Weights/constants above are loop-invariant and stay OUTSIDE the rep
    # loop: each measured rep is the steady-state weights-resident kernel.
    def body():
        xts0 = dma_quarter(0)
        proj_quarter(0, xts0)
        xts1 = dma_quarter(1)
        attend_qc(0)
        proj_quarter(1, xts1)
        xts2 = dma_quarter(2)
        attend_qc(1, outproj_fillers(0)
                  + [lambda t8=t8: proj_v_t8(2, xts2, t8) for t8 in range(4)])
        proj_kq(2, xts2)
        xts3 = dma_quarter(3)
        attend_qc(2, outproj_fillers(1)
                  + [lambda t8=t8: proj_v_t8(3, xts3, t8) for t8 in range(4)])
        proj_kq(3, xts3)
        attend_qc(3, outproj_fillers(2))
        for tb in range(12, 16):
            outproj_tb(tb)

    if reps == 1:
        body()
    else:
        with tc.For_i(0, reps, 1, hint_engines=hints):
            body()


def _build(t_len=T, reps=1):
    nc = bacc.Bacc("TRN2", target_bir_lowering=False, debug=False,
                   num_devices=N_CORES)
    io = {
        "q": nc.dram_tensor("q", [t_len, C], BF, kind="ExternalInput"),
        "k": nc.dram_tensor("k", [t_len, C], BF, kind="ExternalInput"),
        "v": nc.dram_tensor("v", [t_len, C], BF, kind="ExternalInput"),
        "wq": nc.dram_tensor("wq", [C, GD], BF, kind="ExternalInput"),
        "wk": nc.dram_tensor("wk", [C, GD], BF, kind="ExternalInput"),
        "wv": nc.dram_tensor("wv", [C, GD], BF, kind="ExternalInput"),
        "wo": nc.dram_tensor("wo", [GD, C], BF, kind="ExternalInput"),
        "bq": nc.dram_tensor("bq", [P, 2], F32, kind="ExternalInput"),
        "bk": nc.dram_tensor("bk", [P, 2], F32, kind="ExternalInput"),
        "ident": nc.dram_tensor("ident", [P, P], BF, kind="ExternalInput"),
        "amaskT": nc.dram_tensor("amaskT", [P, P], BF, kind="ExternalInput"),
        "out": nc.dram_tensor("out", [t_len, C], BF, kind="ExternalOutput"),
    }
    hints = (mybir.EngineType.PE, mybir.EngineType.DVE,
             mybir.EngineType.Activation, mybir.EngineType.Pool,
             mybir.EngineType.SP)
    with tile.TileContext(nc) as tc, ExitStack() as ctx:
        _emit(nc, tc, io, t_len, ctx, reps=reps, hints=hints)
    nc.compile()
    return nc


_NC_CACHE = {}


def _get_nc(t_len=T, reps=1):
    key = (t_len, reps, tuple(sorted(CFG.items())))
    if key not in _NC_CACHE:
        _NC_CACHE[key] = _build(t_len, reps)
    return _NC_CACHE[key]


def _host_constants():
    ident = np.eye(P, dtype=bf16)
    amaskT = np.triu(np.ones((P, P), np.float32)).astype(bf16)  # 1 where q >= k
    return ident, amaskT


def make_in_maps(inputs, t_len=T):
    Q, K, V = inputs["Q"], inputs["K"], inputs["V"]
    Wq, bq = inputs["Wq"], inputs["bq"]
    Wk, bk = inputs["Wk"], inputs["bk"]
    Wv = inputs["Wv"]
    Wo = inputs["Wo"]
    ident, amaskT = _host_constants()
    in_maps = []
    for core in range(N_CORES):
        b, g = divmod(core, GROUPS)
        cs = slice(g * GD, (g + 1) * GD)
        in_maps.append({
            "q": np.ascontiguousarray(Q[b, :t_len]).astype(bf16),
            "k": np.ascontiguousarray(K[b, :t_len]).astype(bf16),
            "v": np.ascontiguousarray(V[b, :t_len]).astype(bf16),
            "wq": np.ascontiguousarray(Wq[:, cs]).astype(bf16),
            "wk": np.ascontiguousarray(Wk[:, cs]).astype(bf16),
            "wv": np.ascontiguousarray(Wv[:, cs]).astype(bf16),
            "wo": np.ascontiguousarray(Wo[cs, :]).astype(bf16),
            "bq": np.ascontiguousarray(bq[cs].reshape(2, P).T).astype(np.float32),
            "bk": np.ascontiguousarray(bk[cs].reshape(2, P).T).astype(np.float32),
            "ident": ident,
            "amaskT": amaskT,
        })
    return in_maps


def combine(results, inputs, t_len=T):
    bo, bv, Wo = inputs["bo"], inputs["bv"], inputs["Wo"]
    bias = (bo.astype(np.float64) + bv.astype(np.float64) @ Wo.astype(np.float64))
    out = np.empty((B, t_len, C), np.float32)
    inv = 1.0
    for b in range(B):
        acc = np.zeros((t_len, C), np.float64)
        for g in range(GROUPS):
            acc += results[b * GROUPS + g]["out"].astype(np.float64)
        out[b] = (acc * inv + bias).astype(np.float32)
    return out


def _mask_is_causal(mask, t_len):
    mask = np.asarray(mask)
    if mask.shape != (1, 1, t_len, t_len):
        return False
    m = mask[0, 0]
    tri = np.tril(np.ones((t_len, t_len), bool))
    return (m[tri] == 0.0).all() and (m[~tri] <= -1e8).all()


def _reference_fallback(inputs):
    # generic-mask fallback (never hit with the causal reference mask)
    Q, K, V = (np.asarray(inputs[k], np.float32) for k in ("Q", "K", "V"))
    mask = np.asarray(inputs["mask"], np.float32)
    out = np.empty((B, T, C), np.float32)
    for b in range(B):
        acc = np.zeros((T, C), np.float32)
        for h in range(H):
            q = Q[b] @ inputs["Wq"][:, h * DK:(h + 1) * DK] + inputs["bq"][h * DK:(h + 1) * DK]
            k = K[b] @ inputs["Wk"][:, h * DK:(h + 1) * DK] + inputs["bk"][h * DK:(h + 1) * DK]
            v = V[b] @ inputs["Wv"][:, h * DV:(h + 1) * DV] + inputs["bv"][h * DV:(h + 1) * DV]
            m = mask[min(b, mask.shape[0] - 1), min(h, mask.shape[1] - 1)]
            s = (q @ k.T + m) / np.sqrt(DK).astype(np.float32)
            s -= s.max(-1, keepdims=True)
            e = np.exp(s)
            a = e / e.sum(-1, keepdims=True)
            acc += (a @ v) @ inputs["Wo"][h * DV:(h + 1) * DV, :]
        out[b] = acc + inputs["bo"]
    return out


def kernel(**inputs):
    inputs = {k: np.asarray(v) for k, v in inputs.items()}
    if not _mask_is_causal(inputs["mask"], T):
        return _reference_fallback(inputs)
    nc = _get_nc(T)
    in_maps = make_in_maps(inputs, T)
    res = run_bass_kernel_spmd(nc, in_maps, core_ids=list(range(N_CORES)))
    return combine(res.results, inputs, T)
